# revision 47
# baseline (speedup 1.0000x reference)
"""MemoryReader kernel for Trainium2, data-parallel over batch across 8 cores.

Per batch element b (one NeuronCore each):
    mkf = mk[b] as [CK=64, M=4096], qkf = qk[b] as [CK, N=4096]
    aff[m, n] = (2 * mkf.T @ qkf - |mkf[:,m]|^2) / sqrt(CK)
    P = softmax over m
    mem[c, n]  = sum_m mv[b][c, m] * P[m, n]
    out[b] = concat([mem, qv[b]], channel axis)

Device layout (per core), changes vs the 225us baseline:
    - The softmax bias (-asq/8 + EXP_SHIFT, pre-scaled by 4 for the 0.25
      exp scale) rides in partition row 64 of the QK matmul operands:
      mk row 64 carries -asq/2+4 per memory column, qk row 64 carries
      ones. The PSUM then holds ab + 4*bias directly, so every exp uses
      a plain 0 bias. This removes the mkt tensor, its DMA, and the DVE
      square/reduce chain that used to gate the pipeline head.
    - Softmax denominators: supers 0-6 accumulate E with a pairwise DVE
      add tree over pairs 0-13 (fp8 level-1 adds read both operands via
      the two SBUF ports, bf16 2x above) plus a gpsimd fold of pairs
      14/15, finished by TWO bf16 ones-matmuls per super whose PSUM
      result is the denominator replicated across partitions. This
      removes 16 DoubleRow S-matmuls per super (~24us of PE streaming).
      The LAST super keeps per-pair PE S-matmuls - the tree's serial
      tail latency would stretch the output drain.
    - Readout-bank evacuation splits c0/c1 on DVE, c2/c3 on ACT so the
      bank-WAR chain into the next super's first readout is 2 copies
      deep; staging and reciprocals are bf16 so the per-super scaling
      muls run in the DVE 2x mode.
    - Readout contracts over m with fp8 DoubleRow matmuls; exps are
      per-chunk (a batched pair-exp serializes the 3-bank QK PSUM
      recycle chain - measured, not theoretical).
    - Head: inputs stream in 7 DMAs ordered by first use; PE warmup
      matmuls bridge the program-load-to-first-QK window.
    - Tail: the last super scales straight out of PSUM (c3 on gpsimd in
      parallel) with per-chunk output DMAs.
    - PSUM: 3 QK banks + 4 readout accumulators + 1 denominator = 8.
"""

import sys

import numpy as np
import ml_dtypes

B, CK, CV, H, W = 8, 64, 512, 64, 64
M = H * W          # memory positions per batch element
N = H * W          # query positions
NT = 512           # n-super-tile width (columns per softmax pass)
NSUP = N // NT     # 8 n-super-tiles
MCH = M // 128     # 32 m-chunks
NPAIR = MCH // 2   # 16 m-chunk pairs per super
N_CORES = 8
RO_P = 2           # readout trails QK/exp by this many pairs
EXP_SHIFT = 1.0    # fp8 range centering; cancels in normalization
N_WARMUP = 28      # PE pstate warmup matmuls

_CACHE = {}


def _build_program():
    sys.path.insert(0, "/opt/trn_rl_repo")
    from contextlib import ExitStack

    import concourse.tile as tile
    from concourse import bacc, mybir

    dt = mybir.dt
    f32 = dt.float32
    bf16 = dt.bfloat16
    fp8 = dt.float8e4
    DR = mybir.MatmulPerfMode.DoubleRow
    Exp = mybir.ActivationFunctionType.Exp
    Ident = mybir.ActivationFunctionType.Identity

    nc = bacc.Bacc("TRN2", target_bir_lowering=False, debug=False,
                   num_devices=N_CORES)

    # mk rows 0-63 = keys (bf16), row 64 = -asq/2 + 4 (softmax bias,
    # pre-scaled for the 0.25 exp scale), rows 65-127 zero.
    mk_d = nc.dram_tensor("mk", [128, M], bf16, kind="ExternalInput").ap()
    # qk rows 0-63 = queries, row 64 = ones, rest zero.
    qk_d = nc.dram_tensor("qk", [128, N], bf16, kind="ExternalInput").ap()
    # mvt staged host-side as [p, pair, cchunk, 2, 128] so each DoubleRow
    # lhsT block [128, 2, 128] is contiguous in SBUF.
    mvt_d = nc.dram_tensor("mvt", [128, NPAIR * 4 * 2 * 128], fp8,
                           kind="ExternalInput").ap()
    mem_d = nc.dram_tensor("mem", [CV, N], bf16, kind="ExternalOutput").ap()

    with tile.TileContext(nc) as tc, ExitStack() as ctx:
        sing = ctx.enter_context(tc.tile_pool(name="sing", bufs=1))
        e_pool = ctx.enter_context(tc.tile_pool(name="E", bufs=12))
        l1_pool = ctx.enter_context(tc.tile_pool(name="L1", bufs=4))
        l2_pool = ctx.enter_context(tc.tile_pool(name="L2", bufs=3))
        t_pool = ctx.enter_context(tc.tile_pool(name="T", bufs=2))
        rb_pool = ctx.enter_context(tc.tile_pool(name="rb", bufs=2))
        out_pool = ctx.enter_context(tc.tile_pool(name="out", bufs=2))
        qk_ps_pool = ctx.enter_context(
            tc.tile_pool(name="qkps", bufs=3, space="PSUM"))
        ro_ps_pool = ctx.enter_context(
            tc.tile_pool(name="rops", bufs=1, space="PSUM"))
        s_ps_pool = ctx.enter_context(
            tc.tile_pool(name="sps", bufs=1, space="PSUM"))

        # Resident inputs, DMA'd in first-use order, emitted before
        # anything else so the issue stream starts as early as possible.
        mk_sb = sing.tile([128, M], bf16)
        qk_sb = sing.tile([128, N], bf16)
        mvt_sb = sing.tile([128, NPAIR, 4, 2, 128], fp8)

        def mvt_dma(glo, ghi):
            # mvt group g covers pairs 2g..2g+1 (2048 cols each group)
            nc.sync.dma_start(
                out=mvt_sb[:, 2 * glo:2 * ghi, :, :, :],
                in_=mvt_d[:, glo * 2048:ghi * 2048].rearrange(
                    "p (t c i x) -> p t c i x", c=4, i=2, x=128))

        nc.sync.dma_start(out=qk_sb[:, 0:NT], in_=qk_d[:, 0:NT])
        nc.sync.dma_start(out=mk_sb[:, 0:256], in_=mk_d[:, 0:256])
        nc.sync.dma_start(out=mk_sb[:, 256:512], in_=mk_d[:, 256:512])
        mvt_dma(0, 2)
        nc.sync.dma_start(out=mk_sb[:, 512:M], in_=mk_d[:, 512:M])
        mvt_dma(2, 4)
        mvt_dma(4, 8)
        nc.sync.dma_start(out=qk_sb[:, NT:N], in_=qk_d[:, NT:N])

        # PE warmup while the input DMAs stream; a dummy exp pulls the
        # ~1.3us ACT table load into the DMA window instead of the first
        # real exp.
        warm_sb = sing.tile([128, 128], bf16)
        nc.vector.memset(warm_sb[:], 1.0)
        warm_e = sing.tile([128, 128], fp8)
        with nc.allow_low_precision(reason="warmup"):
            nc.scalar.activation(warm_e[:], warm_sb[:], Exp, bias=0.0,
                                 scale=-1.0)
        warm_ps = qk_ps_pool.tile([128, NT], f32, tag="qk_ps", name="warm_ps")
        for w in range(N_WARMUP):
            nc.tensor.matmul(warm_ps[:, 0:128], lhsT=warm_sb[:],
                             rhs=warm_sb[:], start=True, stop=True)

        # fp8 DoubleRow ones weights (denominator matmuls over raw E) and
        # bf16 ones weights (partition-reduce of the DVE add tree's T).
        ones8 = sing.tile([128, 2, 128], fp8)
        nc.vector.memset(ones8[:], 1.0)
        ones16 = sing.tile([128, 128], bf16)
        nc.vector.memset(ones16[:], 1.0)

        state = {}

        def emit_front(i, t):
            # 2 QK matmuls, each followed by its exp (per-chunk exps keep
            # the 3-bank recycle chain slack: a batched pair-exp serializes
            # QK(t+1) behind the full 1.1us exp of pair t).
            nsl = slice(i * NT, (i + 1) * NT)
            e8 = e_pool.tile([128, 2, NT], fp8, tag="E", name=f"e{i}_{t}")
            state[("e", i * NPAIR + t)] = e8
            for h in range(2):
                j = 2 * t + h
                qk_ps = qk_ps_pool.tile([128, NT], f32, tag="qk_ps",
                                        name=f"qkps{i}_{j}")
                nc.tensor.matmul(qk_ps[:],
                                 lhsT=mk_sb[:, j * 128:(j + 1) * 128],
                                 rhs=qk_sb[:, nsl], start=True, stop=True)
                with nc.allow_low_precision(reason="fp8 E by design"):
                    nc.scalar.activation(e8[:, h, :], qk_ps[:],
                                         Exp, bias=0.0, scale=0.25)

        def emit_ro(g):
            # DoubleRow readout matmuls for global pair g.
            i, t = divmod(g, NPAIR)
            e8 = state[("e", g)]
            if t == 0:
                state[("rops", i)] = [
                    ro_ps_pool.tile([128, NT], f32, tag=f"ro{c}",
                                    name=f"ro{c}_{i}") for c in range(4)]
            ro_ps = state[("rops", i)]
            for c in range(4):
                nc.tensor.matmul(
                    ro_ps[c][:],
                    lhsT=mvt_sb[:, t, c, :, :],
                    rhs=e8[:], start=(t == 0), stop=(t == NPAIR - 1),
                    perf_mode=DR)
            if t == NPAIR - 1:
                if i < NSUP - 1:
                    # Evacuate so super i+1's readout can reuse the banks:
                    # c0/c1 on the DVE and c2/c3 on the ACT in parallel, so
                    # the bank-WAR chain into ro(i+1, 0) is ~2 copies deep
                    # instead of 4.
                    ro_ps = state.pop(("rops", i))
                    osb = out_pool.tile([128, 4, NT], bf16, tag="osb",
                                        name=f"osb{i}")
                    state[("osb", i)] = osb
                    with nc.allow_low_precision(reason="bf16 staging"):
                        for c in (0, 1):
                            nc.vector.tensor_copy(osb[:, c, :], ro_ps[c][:])
                        for c in (2, 3):
                            nc.scalar.activation(osb[:, c, :], ro_ps[c][:],
                                                 Ident, bias=0.0, scale=1.0)
                else:
                    # Last super: evacuate just c2/c3 on the (now idle)
                    # ACT so the DVE's final scale chain shortens; c0/c1
                    # scale straight from PSUM.
                    osb = out_pool.tile([128, 4, NT], bf16, tag="osb",
                                        name=f"osb{i}")
                    state[("osb", i)] = osb
                    with nc.allow_low_precision(reason="bf16 staging"):
                        for c in (2, 3):
                            nc.scalar.activation(osb[:, c, :],
                                                 state[("rops", i)][c][:],
                                                 Ident, bias=0.0, scale=1.0)

        def emit_s(g):
            # Last super: per-pair DoubleRow ones-matmul denominator.
            i, t = divmod(g, NPAIR)
            e8 = state[("e", g)]
            if t == 0:
                state[("sps", i)] = s_ps_pool.tile([128, NT], f32,
                                                   tag="sps", name=f"sps{i}")
            s_ps = state[("sps", i)]
            nc.tensor.matmul(s_ps[:], lhsT=ones8[:], rhs=e8[:],
                             start=(t == 0), stop=(t == NPAIR - 1),
                             perf_mode=DR)

        def tree_add(i, dst_key, a_key, b_key, pool, tag):
            # one [128, 2, NT] elementwise add on the DVE
            a = state.pop(a_key) if a_key[0] != "e" else state[a_key]
            b = state.pop(b_key) if b_key[0] != "e" else state[b_key]
            d = pool.tile([128, 2, NT], bf16, tag=tag,
                          name=f"{dst_key[0]}{i}_{dst_key[1]}")
            state[dst_key] = d
            with nc.allow_low_precision(reason="bf16 partial sums"):
                nc.vector.tensor_add(d[:], a[:], b[:])

        def emit_tree(i, t):
            # Pairwise E-sum tree over pairs 0..13 of super i. Pairs 14/15
            # skip the tree (their DoubleRow ones-matmuls join the
            # partition-reduce group) so the DVE has no work left at the
            # super boundary when the readout-bank evacuation must run.
            g0 = i * NPAIR
            if t % 2 == 1 and t <= 13:
                k = t // 2
                tree_add(i, ("l1", k), ("e", g0 + 2 * k), ("e", g0 + 2 * k + 1),
                         l1_pool, "L1")
            if t in (3, 7, 11):
                k = (t - 3) // 4
                tree_add(i, ("l2", k), ("l1", 2 * k), ("l1", 2 * k + 1),
                         l2_pool, "L2")
            if t == 8:
                tree_add(i, ("l3", 0), ("l2", 0), ("l2", 1), l2_pool, "L3")
            if t == 13:
                tree_add(i, ("l3", 1), ("l2", 2), ("l1", 6), l2_pool, "L3b")
            if t == 14:
                tree_add(i, ("l4", 0), ("l3", 0), ("l3", 1), l2_pool, "L4")
                l4 = state.pop(("l4", 0))
                tt = t_pool.tile([128, NT], bf16, tag="T", name=f"t{i}")
                state[("t", i)] = tt
                with nc.allow_low_precision(reason="bf16 partial sums"):
                    nc.vector.tensor_add(tt[:], l4[:, 0, :], l4[:, 1, :])
            if t in (14, 15):
                # pairs 14/15 fold per-pair on the (idle) gpsimd engine so
                # the DVE is free for the boundary evacuation, the
                # s-matmul group stays homogeneous bf16, and each fold is
                # ready ~a slot after its exps (the s-matmuls then land in
                # the boundary's PE bubble as useful filler).
                e8 = state[("e", g0 + t)]
                t2 = t_pool.tile([128, NT], bf16, tag=f"T2{t % 2}",
                                 name=f"t2_{i}_{t}")
                state[("t2", i, t)] = t2
                with nc.allow_low_precision(reason="bf16 partial sums"):
                    nc.gpsimd.tensor_add(t2[:], e8[:, 0, :], e8[:, 1, :])

        def emit_smm(i):
            # Denominator for super i: partition-reduce T (pairs 0-13) and
            # the per-pair folds of pairs 14/15 with three bf16
            # ones-matmuls; the result is the denominator replicated
            # across all 128 partitions.
            tt = state.pop(("t", i))
            t2a = state.pop(("t2", i, 14))
            t2b = state.pop(("t2", i, 15))
            s_ps = s_ps_pool.tile([128, NT], f32, tag="sps", name=f"sps{i}")
            state[("sps", i)] = s_ps
            nc.tensor.matmul(s_ps[:], lhsT=ones16[:], rhs=tt[:],
                             start=True, stop=False)
            nc.tensor.matmul(s_ps[:], lhsT=ones16[:], rhs=t2a[:],
                             start=False, stop=False)
            nc.tensor.matmul(s_ps[:], lhsT=ones16[:], rhs=t2b[:],
                             start=False, stop=True)

        def emit_recip(i):
            s_ps = state.pop(("sps", i))
            rbf = rb_pool.tile([128, NT], f32, tag="rbf", name=f"rbf{i}")
            nc.vector.reciprocal_approx_fast(rbf[:], s_ps[:])
            if i == NSUP - 1:
                # tail-critical: skip the bf16 copy, the PSUM-side muls
                # can't use the DVE 2x mode anyway
                state[("rb", i)] = rbf
                return
            # bf16 copy so the scaling muls hit the DVE 2x mode
            rb = rb_pool.tile([128, NT], bf16, tag="rb", name=f"rb{i}")
            state[("rb", i)] = rb
            with nc.allow_low_precision(reason="bf16 denominators"):
                nc.vector.tensor_copy(rb[:], rbf[:])

        def emit_scale(i):
            rb = state.pop(("rb", i))
            nsl = slice(i * NT, (i + 1) * NT)
            mem_v = mem_d[:, nsl].rearrange("(c p) n -> p c n", p=128)
            obf = out_pool.tile([128, 4, NT], bf16, tag="obf",
                                name=f"obf{i}")
            if i == NSUP - 1:
                # final super: c0/c1 scale straight out of the readout
                # PSUM on the DVE, c2 from its ACT-evacuated bf16 copy on
                # the DVE, c3 on the gpsimd in parallel; DMA per c-chunk.
                ro_ps = state.pop(("rops", i))
                tosb = state.pop(("osb", i))
                with nc.allow_low_precision(reason="bf16 output"):
                    nc.gpsimd.tensor_mul(obf[:, 3, :], tosb[:, 3, :], rb[:])
                nc.gpsimd.dma_start(out=mem_v[:, 3, :], in_=obf[:, 3, :])
                issuers = [nc.sync, nc.scalar, nc.sync]
                for c in range(3):
                    src = ro_ps[c][:] if c < 2 else tosb[:, c, :]
                    with nc.allow_low_precision(reason="bf16 output"):
                        nc.vector.tensor_mul(obf[:, c, :], src, rb[:])
                    issuers[c].dma_start(out=mem_v[:, c, :],
                                         in_=obf[:, c, :])
            else:
                tosb = state.pop(("osb", i))
                for c in range(4):
                    with nc.allow_low_precision(reason="bf16 output"):
                        nc.vector.tensor_mul(obf[:, c, :], tosb[:, c, :],
                                             rb[:])
                # issue from the (idle) gpsimd queue so the sync queue
                # stays clear for input DMAs at the head
                nc.gpsimd.dma_start(out=mem_v[:], in_=obf[:])

        # Pair-granular software pipeline over 128 slots. Readout (and the
        # boundary evacuation) is emitted BEFORE the slot's front so the
        # evacuation copies sit ahead of the next super's L1 adds in the
        # DVE queue. Supers 0-6 take the add-tree denominator path; the
        # last super keeps per-pair PE S-matmuls (slot g+2, not before the
        # previous reciprocal has read the s bank) to keep the tail short.
        TOTAL = NSUP * NPAIR
        LAST = NSUP - 1

        # Last-super S-matmuls trail tightly, but start only after the
        # previous super's smm+recip (slots 5/7) have used the s bank.
        s_sched = {}
        for t in range(NPAIR):
            g = LAST * NPAIR + t
            s_sched.setdefault(max(g + 2, LAST * NPAIR + 5), []).append(g)

        # Readout ramp: the first two pairs of each super (except super 0,
        # which has no preceding evacuation) wait until slot 4 so the
        # previous super's bank evacuation has a 3-slot window.
        ro_sched = {}
        for g in range(TOTAL):
            i = g // NPAIR
            floor = i * NPAIR + 4 if i > 0 else 0
            ro_sched.setdefault(max(g + RO_P, floor), []).append(g)

        last_slot = max(max(s_sched), max(ro_sched)) + 1
        for slot in range(last_slot):
            if slot >= TOTAL:
                # drain region: the s-matmuls feed the reciprocal -> scale
                # -> DMA chain, so they go ahead of the last readouts
                for g in s_sched.get(slot, ()):
                    emit_s(g)
            for r in ro_sched.get(slot, ()):
                emit_ro(r)
            if slot < TOTAL:
                i, t = divmod(slot, NPAIR)
                if i > 0:
                    if t == 2 and i - 1 < LAST:
                        emit_smm(i - 1)
                    if t == 4:
                        emit_recip(i - 1)
                    if t == 7:
                        emit_scale(i - 1)
                emit_front(i, t)
                if i < LAST:
                    emit_tree(i, t)
            if slot < TOTAL:
                for g in s_sched.get(slot, ()):
                    emit_s(g)

        emit_recip(LAST)
        emit_scale(LAST)

    nc.compile()
    return nc


def _get_program():
    if "nc" not in _CACHE:
        _CACHE["nc"] = _build_program()
    return _CACHE["nc"]


def _make_in_maps(mk, qk, mv):
    bf = ml_dtypes.bfloat16
    f8 = ml_dtypes.float8_e4m3
    mk = np.asarray(mk, dtype=np.float32)
    qk = np.asarray(qk, dtype=np.float32)
    mv = np.asarray(mv, dtype=np.float32)
    in_maps = []
    zpad = np.zeros((128 - CK - 1, M), dtype=bf)
    ones_row = np.ones((1, N), dtype=bf)
    zpad_q = np.zeros((128 - CK - 1, N), dtype=bf)
    for b in range(B):
        mk_c = mk[b].reshape(CK, M).astype(bf)
        # softmax bias row: 4 * (-|mk col|^2/8 + EXP_SHIFT), from the
        # bf16-quantized keys so it matches the device-side products.
        asq = np.sum(mk_c.astype(np.float32) ** 2, axis=0, keepdims=True)
        bias_row = (-0.5 * asq + 4.0 * EXP_SHIFT).astype(bf)
        mk_b = np.ascontiguousarray(
            np.concatenate([mk_c, bias_row, zpad], axis=0))
        qk_b = np.ascontiguousarray(
            np.concatenate([qk[b].reshape(CK, N).astype(bf), ones_row,
                            zpad_q], axis=0))
        # mvt[p, (t, c4, i, x)] = mv[b][c4*128 + x, (2t+i)*128 + p]
        mvt_b = np.ascontiguousarray(
            mv[b].reshape(4, 128, NPAIR, 2, 128).transpose(4, 2, 0, 3, 1)
            .reshape(128, NPAIR * 4 * 2 * 128).astype(f8))
        in_maps.append({"mk": mk_b, "qk": qk_b, "mvt": mvt_b})
    return in_maps


def kernel(mk, qk, mv, qv):
    qv = np.asarray(qv, dtype=np.float32)
    nc = _get_program()
    from concourse.bass_utils import run_bass_kernel_spmd

    in_maps = _make_in_maps(mk, qk, mv)
    res = run_bass_kernel_spmd(nc, in_maps, list(range(N_CORES)))
    mem = np.stack([np.asarray(res.results[b]["mem"], dtype=np.float32)
                    for b in range(B)], axis=0)
    mem = mem.reshape(B, CV, H, W)
    return np.concatenate([mem, qv], axis=1)


# revision 48
# speedup vs baseline: 1.0127x; 1.0127x over previous
"""MemoryReader kernel for Trainium2, data-parallel over batch across 8 cores.

Per batch element b (one NeuronCore each):
    mkf = mk[b] as [CK=64, M=4096], qkf = qk[b] as [CK, N=4096]
    aff[m, n] = (2 * mkf.T @ qkf - |mkf[:,m]|^2) / sqrt(CK)
    P = softmax over m
    mem[c, n]  = sum_m mv[b][c, m] * P[m, n]
    out[b] = concat([mem, qv[b]], channel axis)

Device layout (per core), changes vs the 225us baseline:
    - The softmax bias (-asq/8 + EXP_SHIFT, pre-scaled by 4 for the 0.25
      exp scale) rides in partition row 64 of the QK matmul operands:
      mk row 64 carries -asq/2+4 per memory column, qk row 64 carries
      ones. The PSUM then holds ab + 4*bias directly, so every exp uses
      a plain 0 bias. This removes the mkt tensor, its DMA, and the DVE
      square/reduce chain that used to gate the pipeline head.
    - Softmax denominators: supers 0-6 accumulate E with a pairwise DVE
      add tree over pairs 0-13 (fp8 level-1 adds read both operands via
      the two SBUF ports, bf16 2x above) plus a gpsimd fold of pairs
      14/15, finished by TWO bf16 ones-matmuls per super whose PSUM
      result is the denominator replicated across partitions. This
      removes 16 DoubleRow S-matmuls per super (~24us of PE streaming).
      The LAST super keeps per-pair PE S-matmuls - the tree's serial
      tail latency would stretch the output drain.
    - Readout-bank evacuation splits c0/c1 on DVE, c2/c3 on ACT so the
      bank-WAR chain into the next super's first readout is 2 copies
      deep; staging and reciprocals are bf16 so the per-super scaling
      muls run in the DVE 2x mode.
    - Readout contracts over m with fp8 DoubleRow matmuls; exps are
      per-chunk (a batched pair-exp serializes the 3-bank QK PSUM
      recycle chain - measured, not theoretical).
    - Head: inputs stream in 7 DMAs ordered by first use; PE warmup
      matmuls bridge the program-load-to-first-QK window.
    - Tail: the last super scales straight out of PSUM (c3 on gpsimd in
      parallel) with per-chunk output DMAs.
    - PSUM: 3 QK banks + 4 readout accumulators + 1 denominator = 8.
"""

import sys

import numpy as np
import ml_dtypes

B, CK, CV, H, W = 8, 64, 512, 64, 64
M = H * W          # memory positions per batch element
N = H * W          # query positions
NT = 512           # n-super-tile width (columns per softmax pass)
NSUP = N // NT     # 8 n-super-tiles
MCH = M // 128     # 32 m-chunks
NPAIR = MCH // 2   # 16 m-chunk pairs per super
N_CORES = 8
RO_P = 2           # readout trails QK/exp by this many pairs
EXP_SHIFT = 1.0    # fp8 range centering; cancels in normalization
N_WARMUP = 34      # PE pstate warmup matmuls

_CACHE = {}


def _build_program():
    sys.path.insert(0, "/opt/trn_rl_repo")
    from contextlib import ExitStack

    import concourse.tile as tile
    from concourse import bacc, mybir

    dt = mybir.dt
    f32 = dt.float32
    bf16 = dt.bfloat16
    fp8 = dt.float8e4
    DR = mybir.MatmulPerfMode.DoubleRow
    Exp = mybir.ActivationFunctionType.Exp
    Ident = mybir.ActivationFunctionType.Identity

    nc = bacc.Bacc("TRN2", target_bir_lowering=False, debug=False,
                   num_devices=N_CORES)

    # mk rows 0-63 = keys (bf16), row 64 = -asq/2 + 4 (softmax bias,
    # pre-scaled for the 0.25 exp scale), rows 65-127 zero.
    mk_d = nc.dram_tensor("mk", [128, M], bf16, kind="ExternalInput").ap()
    # qk rows 0-63 = queries, row 64 = ones, rest zero.
    qk_d = nc.dram_tensor("qk", [128, N], bf16, kind="ExternalInput").ap()
    # mvt staged host-side as [p, pair, cchunk, 2, 128] so each DoubleRow
    # lhsT block [128, 2, 128] is contiguous in SBUF.
    mvt_d = nc.dram_tensor("mvt", [128, NPAIR * 4 * 2 * 128], fp8,
                           kind="ExternalInput").ap()
    mem_d = nc.dram_tensor("mem", [CV, N], bf16, kind="ExternalOutput").ap()

    with tile.TileContext(nc) as tc, ExitStack() as ctx:
        sing = ctx.enter_context(tc.tile_pool(name="sing", bufs=1))
        e_pool = ctx.enter_context(tc.tile_pool(name="E", bufs=12))
        l1_pool = ctx.enter_context(tc.tile_pool(name="L1", bufs=4))
        l2_pool = ctx.enter_context(tc.tile_pool(name="L2", bufs=3))
        t_pool = ctx.enter_context(tc.tile_pool(name="T", bufs=2))
        rb_pool = ctx.enter_context(tc.tile_pool(name="rb", bufs=2))
        out_pool = ctx.enter_context(tc.tile_pool(name="out", bufs=2))
        qk_ps_pool = ctx.enter_context(
            tc.tile_pool(name="qkps", bufs=3, space="PSUM"))
        ro_ps_pool = ctx.enter_context(
            tc.tile_pool(name="rops", bufs=1, space="PSUM"))
        s_ps_pool = ctx.enter_context(
            tc.tile_pool(name="sps", bufs=1, space="PSUM"))

        # Resident inputs, DMA'd in first-use order, emitted before
        # anything else so the issue stream starts as early as possible.
        mk_sb = sing.tile([128, M], bf16)
        qk_sb = sing.tile([128, N], bf16)
        mvt_sb = sing.tile([128, NPAIR, 4, 2, 128], fp8)

        def mvt_dma(glo, ghi):
            # mvt group g covers pairs 2g..2g+1 (2048 cols each group)
            nc.sync.dma_start(
                out=mvt_sb[:, 2 * glo:2 * ghi, :, :, :],
                in_=mvt_d[:, glo * 2048:ghi * 2048].rearrange(
                    "p (t c i x) -> p t c i x", c=4, i=2, x=128))

        nc.sync.dma_start(out=qk_sb[:, 0:NT], in_=qk_d[:, 0:NT])
        nc.sync.dma_start(out=mk_sb[:, 0:256], in_=mk_d[:, 0:256])
        nc.sync.dma_start(out=mk_sb[:, 256:512], in_=mk_d[:, 256:512])
        mvt_dma(0, 2)
        nc.sync.dma_start(out=mk_sb[:, 512:M], in_=mk_d[:, 512:M])
        mvt_dma(2, 4)
        mvt_dma(4, 8)
        nc.sync.dma_start(out=qk_sb[:, NT:N], in_=qk_d[:, NT:N])

        # PE warmup while the input DMAs stream; a dummy exp pulls the
        # ~1.3us ACT table load into the DMA window instead of the first
        # real exp.
        warm_sb = sing.tile([128, 128], bf16)
        nc.vector.memset(warm_sb[:], 1.0)
        warm_e = sing.tile([128, 128], fp8)
        with nc.allow_low_precision(reason="warmup"):
            nc.scalar.activation(warm_e[:], warm_sb[:], Exp, bias=0.0,
                                 scale=-1.0)
        warm_ps = qk_ps_pool.tile([128, NT], f32, tag="qk_ps", name="warm_ps")
        for w in range(N_WARMUP):
            nc.tensor.matmul(warm_ps[:, 0:128], lhsT=warm_sb[:],
                             rhs=warm_sb[:], start=True, stop=True)

        # fp8 DoubleRow ones weights (denominator matmuls over raw E) and
        # bf16 ones weights (partition-reduce of the DVE add tree's T).
        ones8 = sing.tile([128, 2, 128], fp8)
        nc.vector.memset(ones8[:], 1.0)
        ones16 = sing.tile([128, 128], bf16)
        nc.vector.memset(ones16[:], 1.0)

        state = {}

        def emit_front(i, t):
            # 2 QK matmuls, each followed by its exp (per-chunk exps keep
            # the 3-bank recycle chain slack: a batched pair-exp serializes
            # QK(t+1) behind the full 1.1us exp of pair t).
            nsl = slice(i * NT, (i + 1) * NT)
            e8 = e_pool.tile([128, 2, NT], fp8, tag="E", name=f"e{i}_{t}")
            state[("e", i * NPAIR + t)] = e8
            for h in range(2):
                j = 2 * t + h
                qk_ps = qk_ps_pool.tile([128, NT], f32, tag="qk_ps",
                                        name=f"qkps{i}_{j}")
                nc.tensor.matmul(qk_ps[:],
                                 lhsT=mk_sb[:, j * 128:(j + 1) * 128],
                                 rhs=qk_sb[:, nsl], start=True, stop=True)
                with nc.allow_low_precision(reason="fp8 E by design"):
                    nc.scalar.activation(e8[:, h, :], qk_ps[:],
                                         Exp, bias=0.0, scale=0.25)

        def emit_ro(g):
            # DoubleRow readout matmuls for global pair g.
            i, t = divmod(g, NPAIR)
            e8 = state[("e", g)]
            if t == 0:
                state[("rops", i)] = [
                    ro_ps_pool.tile([128, NT], f32, tag=f"ro{c}",
                                    name=f"ro{c}_{i}") for c in range(4)]
            ro_ps = state[("rops", i)]
            for c in range(4):
                nc.tensor.matmul(
                    ro_ps[c][:],
                    lhsT=mvt_sb[:, t, c, :, :],
                    rhs=e8[:], start=(t == 0), stop=(t == NPAIR - 1),
                    perf_mode=DR)
            if t == NPAIR - 1:
                if i < NSUP - 1:
                    # Evacuate so super i+1's readout can reuse the banks:
                    # c0/c1 on the DVE and c2/c3 on the ACT in parallel, so
                    # the bank-WAR chain into ro(i+1, 0) is ~2 copies deep
                    # instead of 4.
                    ro_ps = state.pop(("rops", i))
                    osb = out_pool.tile([128, 4, NT], bf16, tag="osb",
                                        name=f"osb{i}")
                    state[("osb", i)] = osb
                    with nc.allow_low_precision(reason="bf16 staging"):
                        for c in (0, 1):
                            nc.vector.tensor_copy(osb[:, c, :], ro_ps[c][:])
                        for c in (2, 3):
                            nc.scalar.activation(osb[:, c, :], ro_ps[c][:],
                                                 Ident, bias=0.0, scale=1.0)
                else:
                    # Last super: evacuate just c2/c3 on the (now idle)
                    # ACT so the DVE's final scale chain shortens; c0/c1
                    # scale straight from PSUM.
                    osb = out_pool.tile([128, 4, NT], bf16, tag="osb",
                                        name=f"osb{i}")
                    state[("osb", i)] = osb
                    with nc.allow_low_precision(reason="bf16 staging"):
                        for c in (2, 3):
                            nc.scalar.activation(osb[:, c, :],
                                                 state[("rops", i)][c][:],
                                                 Ident, bias=0.0, scale=1.0)

        def emit_s(g):
            # Last super: per-pair DoubleRow ones-matmul denominator.
            i, t = divmod(g, NPAIR)
            e8 = state[("e", g)]
            if t == 0:
                state[("sps", i)] = s_ps_pool.tile([128, NT], f32,
                                                   tag="sps", name=f"sps{i}")
            s_ps = state[("sps", i)]
            nc.tensor.matmul(s_ps[:], lhsT=ones8[:], rhs=e8[:],
                             start=(t == 0), stop=(t == NPAIR - 1),
                             perf_mode=DR)

        def tree_add(i, dst_key, a_key, b_key, pool, tag):
            # one [128, 2, NT] elementwise add on the DVE
            a = state.pop(a_key) if a_key[0] != "e" else state[a_key]
            b = state.pop(b_key) if b_key[0] != "e" else state[b_key]
            d = pool.tile([128, 2, NT], bf16, tag=tag,
                          name=f"{dst_key[0]}{i}_{dst_key[1]}")
            state[dst_key] = d
            with nc.allow_low_precision(reason="bf16 partial sums"):
                nc.vector.tensor_add(d[:], a[:], b[:])

        def emit_tree(i, t):
            # Pairwise E-sum tree over pairs 0..13 of super i. Pairs 14/15
            # skip the tree (their DoubleRow ones-matmuls join the
            # partition-reduce group) so the DVE has no work left at the
            # super boundary when the readout-bank evacuation must run.
            g0 = i * NPAIR
            if t % 2 == 1 and t <= 13:
                k = t // 2
                tree_add(i, ("l1", k), ("e", g0 + 2 * k), ("e", g0 + 2 * k + 1),
                         l1_pool, "L1")
            if t in (3, 7, 11):
                k = (t - 3) // 4
                tree_add(i, ("l2", k), ("l1", 2 * k), ("l1", 2 * k + 1),
                         l2_pool, "L2")
            if t == 8:
                tree_add(i, ("l3", 0), ("l2", 0), ("l2", 1), l2_pool, "L3")
            if t == 13:
                tree_add(i, ("l3", 1), ("l2", 2), ("l1", 6), l2_pool, "L3b")
            if t == 14:
                tree_add(i, ("l4", 0), ("l3", 0), ("l3", 1), l2_pool, "L4")
                l4 = state.pop(("l4", 0))
                tt = t_pool.tile([128, NT], bf16, tag="T", name=f"t{i}")
                state[("t", i)] = tt
                with nc.allow_low_precision(reason="bf16 partial sums"):
                    nc.vector.tensor_add(tt[:], l4[:, 0, :], l4[:, 1, :])
            if t in (14, 15):
                # pairs 14/15 fold per-pair on the (idle) gpsimd engine so
                # the DVE is free for the boundary evacuation, the
                # s-matmul group stays homogeneous bf16, and each fold is
                # ready ~a slot after its exps (the s-matmuls then land in
                # the boundary's PE bubble as useful filler).
                e8 = state[("e", g0 + t)]
                t2 = t_pool.tile([128, NT], bf16, tag=f"T2{t % 2}",
                                 name=f"t2_{i}_{t}")
                state[("t2", i, t)] = t2
                with nc.allow_low_precision(reason="bf16 partial sums"):
                    nc.gpsimd.tensor_add(t2[:], e8[:, 0, :], e8[:, 1, :])

        def emit_smm(i):
            # Denominator for super i: partition-reduce T (pairs 0-13) and
            # the per-pair folds of pairs 14/15 with three bf16
            # ones-matmuls; the result is the denominator replicated
            # across all 128 partitions.
            tt = state.pop(("t", i))
            t2a = state.pop(("t2", i, 14))
            t2b = state.pop(("t2", i, 15))
            s_ps = s_ps_pool.tile([128, NT], f32, tag="sps", name=f"sps{i}")
            state[("sps", i)] = s_ps
            nc.tensor.matmul(s_ps[:], lhsT=ones16[:], rhs=tt[:],
                             start=True, stop=False)
            nc.tensor.matmul(s_ps[:], lhsT=ones16[:], rhs=t2a[:],
                             start=False, stop=False)
            nc.tensor.matmul(s_ps[:], lhsT=ones16[:], rhs=t2b[:],
                             start=False, stop=True)

        def emit_recip(i):
            s_ps = state.pop(("sps", i))
            rbf = rb_pool.tile([128, NT], f32, tag="rbf", name=f"rbf{i}")
            nc.vector.reciprocal_approx_fast(rbf[:], s_ps[:])
            if i == NSUP - 1:
                # tail-critical: skip the bf16 copy, the PSUM-side muls
                # can't use the DVE 2x mode anyway
                state[("rb", i)] = rbf
                return
            # bf16 copy so the scaling muls hit the DVE 2x mode
            rb = rb_pool.tile([128, NT], bf16, tag="rb", name=f"rb{i}")
            state[("rb", i)] = rb
            with nc.allow_low_precision(reason="bf16 denominators"):
                nc.vector.tensor_copy(rb[:], rbf[:])

        def emit_scale(i):
            rb = state.pop(("rb", i))
            nsl = slice(i * NT, (i + 1) * NT)
            mem_v = mem_d[:, nsl].rearrange("(c p) n -> p c n", p=128)
            obf = out_pool.tile([128, 4, NT], bf16, tag="obf",
                                name=f"obf{i}")
            if i == NSUP - 1:
                # final super: c0/c1 scale straight out of the readout
                # PSUM on the DVE, c2 from its ACT-evacuated bf16 copy on
                # the DVE, c3 on the gpsimd in parallel; DMA per c-chunk.
                ro_ps = state.pop(("rops", i))
                tosb = state.pop(("osb", i))
                with nc.allow_low_precision(reason="bf16 output"):
                    nc.gpsimd.tensor_mul(obf[:, 3, :], tosb[:, 3, :], rb[:])
                nc.gpsimd.dma_start(out=mem_v[:, 3, :], in_=obf[:, 3, :])
                issuers = [nc.sync, nc.scalar, nc.sync]
                for c in range(3):
                    src = ro_ps[c][:] if c < 2 else tosb[:, c, :]
                    with nc.allow_low_precision(reason="bf16 output"):
                        nc.vector.tensor_mul(obf[:, c, :], src, rb[:])
                    issuers[c].dma_start(out=mem_v[:, c, :],
                                         in_=obf[:, c, :])
            else:
                tosb = state.pop(("osb", i))
                for c in range(4):
                    with nc.allow_low_precision(reason="bf16 output"):
                        nc.vector.tensor_mul(obf[:, c, :], tosb[:, c, :],
                                             rb[:])
                # issue from the (idle) gpsimd queue so the sync queue
                # stays clear for input DMAs at the head
                nc.gpsimd.dma_start(out=mem_v[:], in_=obf[:])

        # Pair-granular software pipeline over 128 slots. Readout (and the
        # boundary evacuation) is emitted BEFORE the slot's front so the
        # evacuation copies sit ahead of the next super's L1 adds in the
        # DVE queue. Supers 0-6 take the add-tree denominator path; the
        # last super keeps per-pair PE S-matmuls (slot g+2, not before the
        # previous reciprocal has read the s bank) to keep the tail short.
        TOTAL = NSUP * NPAIR
        LAST = NSUP - 1

        # Last-super S-matmuls trail tightly, but start only after the
        # previous super's smm+recip (slots 5/7) have used the s bank.
        s_sched = {}
        for t in range(NPAIR):
            g = LAST * NPAIR + t
            s_sched.setdefault(max(g + 2, LAST * NPAIR + 5), []).append(g)

        # Readout ramp: the first two pairs of each super (except super 0,
        # which has no preceding evacuation) wait until slot 4 so the
        # previous super's bank evacuation has a 3-slot window.
        ro_sched = {}
        for g in range(TOTAL):
            i = g // NPAIR
            floor = i * NPAIR + 4 if i > 0 else 0
            ro_sched.setdefault(max(g + RO_P, floor), []).append(g)

        last_slot = max(max(s_sched), max(ro_sched)) + 1
        for slot in range(last_slot):
            if slot >= TOTAL:
                # drain region: the s-matmuls feed the reciprocal -> scale
                # -> DMA chain, so they go ahead of the last readouts
                for g in s_sched.get(slot, ()):
                    emit_s(g)
            for r in ro_sched.get(slot, ()):
                emit_ro(r)
            if slot < TOTAL:
                i, t = divmod(slot, NPAIR)
                if i > 0:
                    if t == 2 and i - 1 < LAST:
                        emit_smm(i - 1)
                    if t == 4:
                        emit_recip(i - 1)
                    if t == 7:
                        emit_scale(i - 1)
                emit_front(i, t)
                if i < LAST:
                    emit_tree(i, t)
            if slot < TOTAL:
                for g in s_sched.get(slot, ()):
                    emit_s(g)

        emit_recip(LAST)
        emit_scale(LAST)

    nc.compile()
    return nc


def _get_program():
    if "nc" not in _CACHE:
        _CACHE["nc"] = _build_program()
    return _CACHE["nc"]


def _make_in_maps(mk, qk, mv):
    bf = ml_dtypes.bfloat16
    f8 = ml_dtypes.float8_e4m3
    mk = np.asarray(mk, dtype=np.float32)
    qk = np.asarray(qk, dtype=np.float32)
    mv = np.asarray(mv, dtype=np.float32)
    in_maps = []
    zpad = np.zeros((128 - CK - 1, M), dtype=bf)
    ones_row = np.ones((1, N), dtype=bf)
    zpad_q = np.zeros((128 - CK - 1, N), dtype=bf)
    for b in range(B):
        mk_c = mk[b].reshape(CK, M).astype(bf)
        # softmax bias row: 4 * (-|mk col|^2/8 + EXP_SHIFT), from the
        # bf16-quantized keys so it matches the device-side products.
        asq = np.sum(mk_c.astype(np.float32) ** 2, axis=0, keepdims=True)
        bias_row = (-0.5 * asq + 4.0 * EXP_SHIFT).astype(bf)
        mk_b = np.ascontiguousarray(
            np.concatenate([mk_c, bias_row, zpad], axis=0))
        qk_b = np.ascontiguousarray(
            np.concatenate([qk[b].reshape(CK, N).astype(bf), ones_row,
                            zpad_q], axis=0))
        # mvt[p, (t, c4, i, x)] = mv[b][c4*128 + x, (2t+i)*128 + p]
        mvt_b = np.ascontiguousarray(
            mv[b].reshape(4, 128, NPAIR, 2, 128).transpose(4, 2, 0, 3, 1)
            .reshape(128, NPAIR * 4 * 2 * 128).astype(f8))
        in_maps.append({"mk": mk_b, "qk": qk_b, "mvt": mvt_b})
    return in_maps


def kernel(mk, qk, mv, qv):
    qv = np.asarray(qv, dtype=np.float32)
    nc = _get_program()
    from concourse.bass_utils import run_bass_kernel_spmd

    in_maps = _make_in_maps(mk, qk, mv)
    res = run_bass_kernel_spmd(nc, in_maps, list(range(N_CORES)))
    mem = np.stack([np.asarray(res.results[b]["mem"], dtype=np.float32)
                    for b in range(B)], axis=0)
    mem = mem.reshape(B, CV, H, W)
    return np.concatenate([mem, qv], axis=1)
